# revision 68
# baseline (speedup 1.0000x reference)
"""PixelAttention Trainium2 kernel (v2).

Data-parallel: one image per NeuronCore. Per core:
    seq  = image.reshape(C, T).T            # [T, C], T = 32*32
    kqv  = seq @ w_kqv + b_kqv
    per-head causal attention (8 heads, head_dim 32), softmax over keys
    out  = mix(attn) + b_mix + image

Key design points (vs v1 baseline at ~87us):
  - exp is split across ScalarE (true Exp) and DVE (one-instruction
    Schraudolph fast-exp: int16(L*2^7/ln2 + (127*128-5.5)) bitcast to bf16,
    max rel err ~3.3%, end-to-end ~2e-3).
  - causal diag masking: ScalarE-assigned tiles get -60 added to the masked
    triangle via an eye-matmul PSUM accumulate (PE); DVE-assigned tiles fold
    the mask into the fast-exp via scalar_tensor_tensor with a const tile.
  - AV matmuls use M=64 weights [V | ones | zeros*31] so all 128 PSUM rows
    are written (denominator at rows 32/96, junk rows exactly zero); the
    division then runs directly on the PO layout, no compaction DMAs.
  - kqv biases ride on the psum->sbuf mover ops; v-bias is folded into the
    mix bias host-side (bm_eff = b_mix + bv @ w_mix).
  - software pipeline: logits(st) issue before AV(st-1) so the PE never
    stalls on exp and stays at full pstate.
"""

import numpy as np
import ml_dtypes

import concourse.bass as bass
import concourse.tile as tile
from concourse import bacc, mybir
from concourse.bass_utils import run_bass_kernel_spmd

BF = ml_dtypes.bfloat16
T, C, H, D = 1024, 256, 8, 32
N_CORES = 8

EXP_A = float(2.0**7 / np.log(2.0))        # 184.66496
EXP_B = float(127 * 128 - 5.5)             # 16250.5
MASK60 = 60.0                              # causal mask additive offset

_CACHE = {}


def _unit_engine(c, g, st, pair):
    """Which engine computes exp for unit (c, g, st, pair).

    'A' = ScalarE true exp (diag tiles masked by Pool tri-multiply on E),
    'D' = DVE fast-exp (diag mask folded into scalar_tensor_tensor).
    """
    return "A" if pair == 0 else "D"


def _build_nc():
    f32 = mybir.dt.float32
    bf16 = mybir.dt.bfloat16
    i16 = mybir.dt.int16
    EXP = mybir.ActivationFunctionType.Exp
    IDENT = mybir.ActivationFunctionType.Identity
    ADD = mybir.AluOpType.add
    MULT = mybir.AluOpType.mult

    nc = bacc.Bacc("TRN2", target_bir_lowering=False, debug=False)

    def din(name, shape, dt):
        return nc.dram_tensor(name, shape, dt, kind="ExternalInput").ap()

    x_bf = din("x_bf", [128, 2, 2, 512], bf16)  # [p, tchunk, chalf, t]
    wk = din("wk", [128, 2, 256], bf16)
    wq = din("wq", [128, 2, 256], bf16)  # pre-scaled by 1/sqrt(D)
    wv = din("wv", [128, 2, 256], bf16)
    bjt = din("bjt", [4, 128], f32)    # bk0, bk1, bq0, bq1 (q pre-scaled)
    bm2 = din("bm2", [2, 128], f32)    # b_mix + bv @ w_mix
    mbf = din("mbf", [128, 512], f32)    # col<128: B - 60*A if s>t else B; else B
    selp = din("selp", [128, 128], bf16)  # denom-row broadcast selector
    wmp = din("wmp", [128, 4, 256], bf16)  # po-row layout mix weights
    y = nc.dram_tensor("y", [C, T], bf16, kind="ExternalOutput").ap()

    with tile.TileContext(nc) as tc:
        with (
            tc.tile_pool(name="consts", bufs=1) as consts,
            tc.tile_pool(name="sb", bufs=4) as sb,
            tc.tile_pool(name="lpp", bufs=3, space="PSUM") as lp_pool,
            tc.tile_pool(name="pop", bufs=2, space="PSUM") as po_pool,
        ):
            # ---------------- input DMAs (critical-path order) -------------
            # xb[p, tc, a, t]: per-partition 4KB contiguous lines in DRAM.
            # tc0 split into 4 chunks so the (column-chunked) first
            # projections start as early as possible; rings ordered by
            # deadline (3 rings share ~150GB/s while all active).
            xb = consts.tile([128, 2, 2, 512], bf16)
            nc.sync.dma_start(out=xb[:, 0, 0, 0:256], in_=x_bf[:, 0, 0, 0:256])
            nc.sync.dma_start(out=xb[:, 0, 1, 0:256], in_=x_bf[:, 0, 1, 0:256])
            nc.sync.dma_start(out=xb[:, 0, 0, 256:512],
                              in_=x_bf[:, 0, 0, 256:512])
            nc.sync.dma_start(out=xb[:, 0, 1, 256:512],
                              in_=x_bf[:, 0, 1, 256:512])
            w_tiles = {}
            for name in ("wk", "wq", "wv"):
                w_tiles[name] = consts.tile([128, 2, 256], bf16,
                                            tag=name, name=name)
            nc.sync.dma_start(out=w_tiles["wv"], in_=wv)
            # wk j0 rides the scalar HWDGE queue (lower latency, idle early)
            # so the first projection isn't gated on the big gpsimd loads.
            # bjt AFTER it: its 128 tiny descriptors would block the ring.
            nc.scalar.dma_start(out=w_tiles["wk"][:, 0, 0:128],
                                in_=wk[:, 0, 0:128])
            nc.scalar.dma_start(out=w_tiles["wk"][:, 1, 0:128],
                                in_=wk[:, 1, 0:128])
            bjt_sb = consts.tile([128, 4], f32)
            nc.scalar.dma_start(out=bjt_sb, in_=bjt.rearrange("a p -> p a"))
            nc.gpsimd.dma_start(out=w_tiles["wq"][:, :, 0:128],
                                in_=wq[:, :, 0:128])
            nc.gpsimd.dma_start(out=w_tiles["wk"][:, :, 128:256],
                                in_=wk[:, :, 128:256])
            nc.gpsimd.dma_start(out=w_tiles["wq"][:, :, 128:256],
                                in_=wq[:, :, 128:256])
            mbf_sb = consts.tile([128, 512], f32)
            nc.scalar.dma_start(out=mbf_sb, in_=mbf)
            bm2_sb = consts.tile([128, 2], f32)
            nc.scalar.dma_start(out=bm2_sb, in_=bm2.rearrange("a p -> p a"))
            nc.sync.dma_start(out=xb[:, 1], in_=x_bf[:, 1])
            selp_sb = consts.tile([128, 128], bf16)
            nc.sync.dma_start(out=selp_sb, in_=selp)
            wmp_sb = consts.tile([128, 4, 256], bf16)
            nc.sync.dma_start(out=wmp_sb, in_=wmp)

            # warm the exp table before attention needs it
            warm = consts.tile([128, 1], f32, tag="warm", name="warm")
            nc.scalar.activation(out=warm, in_=bjt_sb[:, 0:1], func=EXP)

            qkT = consts.tile([128, 4, T], bf16)  # 0-1: kT g0/g1, 2-3: qT
            vsb = consts.tile([128, 8, H, 33], bf16)  # [p, st, head, V|1]
            nc.vector.memset(vsb[:, :, :, 32:33], 1.0)
            attnT = consts.tile([128, 4, T], bf16)    # [po-row, 2g+pair, t]

            # ---------------- projections --------------------------------
            def proj_group(dst, wname, jl, tch, mover, chunked=False):
                w_s = w_tiles[wname]
                js = slice(jl * 128, (jl + 1) * 128)
                ts_ = slice(tch * 512, (tch + 1) * 512)
                p = lp_pool.tile([128, 2, 512], f32, tag="lp", name="pp")[:, 0, :]
                if chunked:
                    # column-chunked so the first matmuls can start before
                    # the whole x t-half has landed
                    for ch in range(2):
                        cc = slice(256 * ch, 256 * (ch + 1))
                        nc.tensor.matmul(out=p[:, cc], lhsT=w_s[:, 0, js],
                                         rhs=xb[:, tch, 0, cc],
                                         start=True, stop=False,
                                         skip_group_check=True)
                        nc.tensor.matmul(out=p[:, cc], lhsT=w_s[:, 1, js],
                                         rhs=xb[:, tch, 1, cc],
                                         start=False, stop=True,
                                         skip_group_check=True)
                else:
                    nc.tensor.matmul(out=p, lhsT=w_s[:, 0, js],
                                     rhs=xb[:, tch, 0],
                                     start=True, stop=False)
                    nc.tensor.matmul(out=p, lhsT=w_s[:, 1, js],
                                     rhs=xb[:, tch, 1],
                                     start=False, stop=True)
                if mover == "A":
                    nc.scalar.activation(out=qkT[:, dst, ts_], in_=p,
                                         func=IDENT,
                                         bias=bjt_sb[:, dst:dst + 1])
                else:
                    eng = nc.vector if mover == "D" else nc.gpsimd
                    eng.tensor_scalar(out=qkT[:, dst, ts_], in0=p,
                                      scalar1=bjt_sb[:, dst:dst + 1],
                                      scalar2=None, op0=ADD)

            def v_group(st, mover):
                tc, tr = st // 4, st % 4
                ss = slice(tr * 128, (tr + 1) * 128)
                p = lp_pool.tile([128, 2, 512], f32, tag="lp",
                                 name="pv")[:, 0, 0:256]
                nc.tensor.matmul(out=p, lhsT=xb[:, tc, 0, ss],
                                 rhs=w_tiles["wv"][:, 0, :],
                                 start=True, stop=False)
                nc.tensor.matmul(out=p, lhsT=xb[:, tc, 1, ss],
                                 rhs=w_tiles["wv"][:, 1, :],
                                 start=False, stop=True)
                src = p.rearrange("p (h e) -> p h e", e=32)
                dstv = vsb[:, st, :, 0:32]
                if mover == "A":
                    nc.scalar.copy(out=dstv, in_=src)
                elif mover == "D":
                    nc.vector.tensor_copy(out=dstv, in_=src)
                else:
                    nc.gpsimd.tensor_copy(out=dstv, in_=src)

            # head phase: only what c0/g0 needs
            proj_group(0, "wk", 0, 0, "A", chunked=True)   # kT g0, t 0:512
            proj_group(2, "wq", 0, 0, "A", chunked=True)   # qT g0, t 0:512
            for st in range(4):
                v_group(st, "A" if st % 2 == 0 else "D")

            # woven work lists: (c, g, st) -> list of thunks
            weave = {}

            def add_weave(c, g, st, fn):
                weave.setdefault((c, g, st), []).append(fn)

            add_weave(0, 0, 0, lambda: proj_group(1, "wk", 1, 0, "A"))
            add_weave(0, 0, 1, lambda: proj_group(3, "wq", 1, 0, "A"))
            add_weave(0, 0, 2, lambda: proj_group(0, "wk", 0, 1, "A"))
            add_weave(0, 0, 3, lambda: proj_group(2, "wq", 0, 1, "A"))
            add_weave(0, 1, 0, lambda: proj_group(1, "wk", 1, 1, "A"))
            add_weave(0, 1, 1, lambda: proj_group(3, "wq", 1, 1, "A"))
            for st in range(4, 8):
                m = "A" if st % 2 == 0 else "D"
                add_weave(0, 1, st - 4, lambda st=st, m=m: v_group(st, m))

            # ---------------- attention ----------------------------------
            ouPO = {}   # (c, g) -> [128, 2, 512] bf16 sbuf

            def drain_po(c, g, po, q0=0, q1=512):
                if (c, g) not in ouPO:
                    ouPO[(c, g)] = sb.tile([128, 2, 512], bf16, tag="oup",
                                           name=f"ou{c}{g}", bufs=4)
                t_ = ouPO[(c, g)]
                nc.vector.tensor_copy(out=t_[:, 0, q0:q1],
                                      in_=po[0][:, q0:q1])
                nc.scalar.copy(out=t_[:, 1, q0:q1], in_=po[1][:, q0:q1])

            def div_cg(c, g, q0=0, q1=512):
                # broadcast denominator rows (32/96) to all partitions via
                # the selector matmul (one per pair/bank), reciprocal on all
                # 128 lanes, then one tensor_mul for both pairs.
                w = q1 - q0
                bc = lp_pool.tile([128, 2, 512], f32, tag="lp", name="bc")
                if w <= 128:
                    # both pairs fit one PSUM bank: single broadcast matmul
                    nc.tensor.matmul(out=bc[:, :, q0:q1], lhsT=selp_sb,
                                     rhs=ouPO[(c, g)][:, :, q0:q1],
                                     start=True, stop=True,
                                     skip_group_check=True)
                else:
                    for pair in range(2):
                        nc.tensor.matmul(out=bc[:, pair, q0:q1],
                                         lhsT=selp_sb,
                                         rhs=ouPO[(c, g)][:, pair, q0:q1],
                                         start=True, stop=True,
                                         skip_group_check=True)
                rcf = sb.tile([128, 2, 512], f32, tag="rcf", name="rcf",
                              bufs=4)
                nc.vector.reciprocal_approx_fast(
                    out=rcf[:, :, 0:w], in_=bc[:, :, q0:q1])
                nc.vector.tensor_mul(
                    out=attnT[:, 2 * g:2 * g + 2, c * 512 + q0:c * 512 + q1],
                    in0=ouPO[(c, g)][:, :, q0:q1], in1=rcf[:, :, 0:w])

            mp_held = {}

            def mix_part(c, c2t, gps, start, stop, q0=0, q1=512):
                cs = slice(c * 512 + q0, c * 512 + q1)
                c2s = slice(c2t * 128, (c2t + 1) * 128)
                if (c, c2t) not in mp_held:
                    mp_held[(c, c2t)] = lp_pool.tile(
                        [128, 2, 512], f32, tag="lp", name="mp")[:, 0, :]
                mp = mp_held[(c, c2t)]
                for i, gp in enumerate(gps):
                    nc.tensor.matmul(out=mp[:, q0:q1],
                                     lhsT=wmp_sb[:, gp, c2s],
                                     rhs=attnT[:, gp, cs],
                                     start=(start and i == 0),
                                     stop=(stop and i == len(gps) - 1),
                                     skip_group_check=True)

            def mix_fin(c, c2t, q0=0, q1=512, eng="A"):
                mp = mp_held[(c, c2t)]
                os_ = sb.tile([128, 512], bf16, tag="os", name="os")
                nc.vector.scalar_tensor_tensor(
                    out=os_[:, q0:q1], in0=mp[:, q0:q1],
                    scalar=bm2_sb[:, c2t:c2t + 1],
                    in1=xb[:, c, c2t, q0:q1], op0=ADD, op1=ADD)
                y_r = y.rearrange("(a p) t -> p a t", p=128)
                qe1 = (nc.sync, nc.scalar)[c2t]
                qe2 = (nc.gpsimd, nc.sync)[c2t]
                if q1 - q0 >= 256:
                    qm = (q0 + q1) // 2
                    qe1.dma_start(out=y_r[:, c2t, c * 512 + q0:c * 512 + qm],
                                  in_=os_[:, q0:qm])
                    qe2.dma_start(out=y_r[:, c2t, c * 512 + qm:c * 512 + q1],
                                  in_=os_[:, qm:q1])
                else:
                    qe1.dma_start(out=y_r[:, c2t, c * 512 + q0:c * 512 + q1],
                                  in_=os_[:, q0:q1])

            def mix_c(c, c2t):
                mix_part(c, c2t, (0, 1, 2, 3), True, True)
                mix_fin(c, c2t)

            # drains + division woven two slots later than strictly needed
            # so they don't sit between the latency-critical early-group
            # exps on ACT/DVE; c0's mix woven into (c1, g1)'s late slots.
            po_of = {}
            add_weave(0, 1, 3, lambda: drain_po(0, 0, po_of[(0, 0)]))
            add_weave(1, 0, 1, lambda: div_cg(0, 0))
            add_weave(1, 0, 3, lambda: drain_po(0, 1, po_of[(0, 1)]))
            add_weave(1, 0, 5, lambda: div_cg(0, 1))
            add_weave(1, 0, 6, lambda: mix_c(0, 0))
            add_weave(1, 0, 7, lambda: mix_c(0, 1))
            add_weave(1, 1, 3, lambda: drain_po(1, 0, po_of[(1, 0)]))
            add_weave(1, 1, 5, lambda: div_cg(1, 0))
            add_weave(1, 1, 7, lambda: mix_part(1, 0, (0, 1), True, False))

            for c in range(2):
                n_st = 4 + 4 * c
                for g in range(2):
                    po = {
                        0: po_pool.tile([128, 512], f32, tag="po", name="po0"),
                        1: po_pool.tile([128, 512], f32, tag="po", name="po1"),
                    }
                    e_tiles = {}
                    LAG = 3
                    for st in range(n_st + LAG):
                        if st < n_st:
                            tlo_r = 128 * st - 512 * c
                            diag = tlo_r >= 0
                            tlo = max(tlo_r, 0)
                            for pair in (1, 0):
                                eng = _unit_engine(c, g, st, pair)
                                lp = lp_pool.tile([128, 2, 512], f32,
                                                  tag="lp", name="lp")
                                for h2 in range(2):
                                    hl = 2 * pair + h2
                                    rp = 32 * hl
                                    kT_l = qkT[rp:rp + 32, g,
                                               st * 128:(st + 1) * 128]
                                    qg = qkT[rp:rp + 32, 2 + g, :]
                                    nc.tensor.matmul(
                                        out=lp[:, h2, tlo:512], lhsT=kT_l,
                                        rhs=qg[:, c * 512 + tlo:(c + 1) * 512],
                                        start=True, stop=True,
                                        tile_position=(rp, 0),
                                    )
                                E = sb.tile([128, 2, 512], bf16, tag="E",
                                            name="E", bufs=8)
                                if eng == "A":
                                    nc.scalar.activation(
                                        out=E[:, :, tlo:512],
                                        in_=lp[:, :, tlo:512], func=EXP)
                                    if diag:
                                        # zero the s>t triangle of the diag
                                        # block: keep where (t - s) >= 0
                                        nc.gpsimd.affine_select(
                                            out=E[:, :, tlo:tlo + 128],
                                            in_=E[:, :, tlo:tlo + 128],
                                            pattern=[[0, 2], [1, 128]],
                                            compare_op=mybir.AluOpType.is_ge,
                                            fill=0.0,
                                            base=0,
                                            channel_multiplier=-1,
                                        )
                                else:
                                    E16 = E.bitcast(i16)
                                    if diag:
                                        w_ = 512 - tlo
                                        mb_b = bass.AP(
                                            tensor=mbf_sb.tensor,
                                            offset=mbf_sb.offset,
                                            ap=[list(mbf_sb.ap[0]), [0, 2],
                                                [1, w_]],
                                        )
                                        nc.vector.scalar_tensor_tensor(
                                            out=E16[:, :, tlo:512],
                                            in0=lp[:, :, tlo:512],
                                            scalar=EXP_A, in1=mb_b,
                                            op0=MULT, op1=ADD)
                                    else:
                                        nc.vector.tensor_scalar(
                                            out=E16[:, :, tlo:512],
                                            in0=lp[:, :, tlo:512],
                                            scalar1=EXP_A, scalar2=EXP_B,
                                            op0=MULT, op1=ADD)
                                e_tiles[(st, pair)] = E
                            for fn in weave.get((c, g, st), []):
                                fn()
                        if st >= LAG:
                            stp = st - LAG
                            tlo_p = max(128 * stp - 512 * c, 0)
                            for pair in (1, 0):
                                E = e_tiles.pop((stp, pair))
                                for h2 in range(2):
                                    hl = 2 * pair + h2
                                    h = 4 * g + hl
                                    nc.tensor.matmul(
                                        out=po[pair][64 * h2:64 * h2 + 33,
                                                     tlo_p:512],
                                        lhsT=vsb[:, stp, h, :],
                                        rhs=E[:, h2, tlo_p:512],
                                        start=(stp == 0),
                                        stop=(stp == n_st - 1),
                                        skip_group_check=True,
                                        tile_position=(0, 64 * h2),
                                    )
                    po_of[(c, g)] = po

            # ---------------- tail: c1g1 division + c1 mix ---------------
            # AV st7 only touches query cols 384:512 of the (1,1) window, so
            # cols 0:384 ("A") drain/divide/mix while st7 is still in
            # flight; only the 128-wide "B" chunk rides the critical tail.
            mix_part(1, 1, (0, 1), True, False)
            drain_po(1, 1, po_of[(1, 1)], 0, 384)
            div_cg(1, 1, 0, 384)
            mix_part(1, 0, (2, 3), False, False, 0, 384)
            mix_part(1, 1, (2, 3), False, False, 0, 384)
            mix_fin(1, 0, 0, 384)
            mix_fin(1, 1, 0, 384)
            drain_po(1, 1, po_of[(1, 1)], 384, 512)
            div_cg(1, 1, 384, 512)
            mix_part(1, 0, (2, 3), False, True, 384, 512)
            mix_part(1, 1, (2, 3), False, True, 384, 512)
            mix_fin(1, 0, 384, 512, eng="D")
            mix_fin(1, 1, 384, 512, eng="D")

    nc.compile()
    return nc


def _host_inputs(image, w_kqv, b_kqv, w_mix, b_mix):
    s = np.float32(1.0 / np.sqrt(D))
    wk = w_kqv[:, :256]
    wq = w_kqv[:, 256:512] * s
    wv = w_kqv[:, 512:]
    bk = b_kqv[:256].astype(np.float32)
    bq = (b_kqv[256:512] * s).astype(np.float32)
    bv = b_kqv[512:].astype(np.float32)
    bjt = np.stack([bk[0:128], bk[128:256], bq[0:128], bq[128:256]])
    bm_eff = (np.asarray(b_mix, np.float32)
              + bv @ np.asarray(w_mix, np.float32))
    bm2 = bm_eff.reshape(2, 128)

    idx = np.arange(128)
    mask = (idx[:, None] > idx[None, :]).astype(np.float32)  # s > t
    mbf = np.full((128, 512), EXP_B, np.float32)
    mbf[:, 0:128] = EXP_B - (MASK60 * EXP_A) * mask

    # selp[po_row, m]: broadcast denominator row 32 (h2=0) to partitions
    # 0-63, row 96 (h2=1) to partitions 64-127.
    selp = np.zeros((128, 128), np.float32)
    selp[32, 0:64] = 1.0
    selp[96, 64:128] = 1.0
    # wmp[po_row, gp, out_chan]
    wmp = np.zeros((128, 4, 256), np.float32)
    wm = np.asarray(w_mix, np.float32)
    for g in range(2):
        for pair in range(2):
            gp = 2 * g + pair
            for h2 in range(2):
                in0 = 128 * g + 32 * (2 * pair + h2)
                wmp[64 * h2:64 * h2 + 32, gp, :] = wm[in0:in0 + 32, :]

    def play(w):  # [C, 256] -> [p, a, j] contiguous
        return np.ascontiguousarray(
            np.asarray(w, np.float32).reshape(2, 128, 256).transpose(1, 0, 2)
        ).astype(BF)

    common = {
        "wk": play(wk),
        "wq": play(wq),
        "wv": play(wv),
        "wmp": wmp.astype(BF),
        "bjt": np.ascontiguousarray(bjt),
        "bm2": np.ascontiguousarray(bm2),
        "mbf": mbf,
        "selp": selp.astype(BF),
    }
    in_maps = []
    for i in range(N_CORES):
        x = np.asarray(image[i].reshape(C, T), np.float32)
        # [p, tc, a, t]: per-partition 4KB contiguous DMA lines
        x4 = np.ascontiguousarray(
            x.reshape(2, 128, 2, 512).transpose(1, 2, 0, 3)).astype(BF)
        in_maps.append({**common, "x_bf": x4})
    return in_maps


def _run(inputs, trace=False):
    if "nc" not in _CACHE:
        _CACHE["nc"] = _build_nc()
    nc = _CACHE["nc"]
    in_maps = _host_inputs(
        np.asarray(inputs["image"], np.float32),
        np.asarray(inputs["w_kqv"], np.float32),
        np.asarray(inputs["b_kqv"], np.float32),
        np.asarray(inputs["w_mix"], np.float32),
        np.asarray(inputs["b_mix"], np.float32),
    )
    res = run_bass_kernel_spmd(nc, in_maps, list(range(N_CORES)), trace=trace)
    out = np.stack(
        [np.asarray(res.results[i]["y"]).reshape(C, 32, 32) for i in range(N_CORES)]
    ).astype(np.float32)
    return out, res


def kernel(**inputs):
    out, _ = _run(inputs, trace=False)
    return out



# revision 69
# speedup vs baseline: 1.1902x; 1.1902x over previous
"""PixelAttention Trainium2 kernel (v2).

Data-parallel: one image per NeuronCore. Per core:
    seq  = image.reshape(C, T).T            # [T, C], T = 32*32
    kqv  = seq @ w_kqv + b_kqv
    per-head causal attention (8 heads, head_dim 32), softmax over keys
    out  = mix(attn) + b_mix + image

Key design points (vs v1 baseline at ~87us):
  - exp is split across ScalarE (true Exp) and DVE (one-instruction
    Schraudolph fast-exp: int16(L*2^7/ln2 + (127*128-5.5)) bitcast to bf16,
    max rel err ~3.3%, end-to-end ~2e-3).
  - causal diag masking: ScalarE-assigned tiles get -60 added to the masked
    triangle via an eye-matmul PSUM accumulate (PE); DVE-assigned tiles fold
    the mask into the fast-exp via scalar_tensor_tensor with a const tile.
  - AV matmuls use M=64 weights [V | ones | zeros*31] so all 128 PSUM rows
    are written (denominator at rows 32/96, junk rows exactly zero); the
    division then runs directly on the PO layout, no compaction DMAs.
  - kqv biases ride on the psum->sbuf mover ops; v-bias is folded into the
    mix bias host-side (bm_eff = b_mix + bv @ w_mix).
  - software pipeline: logits(st) issue before AV(st-1) so the PE never
    stalls on exp and stays at full pstate.
"""

import numpy as np
import ml_dtypes

import concourse.bass as bass
import concourse.tile as tile
from concourse import bacc, mybir
from concourse.bass_utils import run_bass_kernel_spmd

BF = ml_dtypes.bfloat16
T, C, H, D = 1024, 256, 8, 32
N_CORES = 8

EXP_A = float(2.0**7 / np.log(2.0))        # 184.66496
EXP_B = float(127 * 128 - 5.5)             # 16250.5
MASK60 = 60.0                              # causal mask additive offset

_CACHE = {}


def _unit_engine(c, g, st, pair):
    """Which engine computes exp for unit (c, g, st, pair).

    'A' = ScalarE true exp (diag tiles masked by Pool tri-multiply on E),
    'D' = DVE fast-exp (diag mask folded into scalar_tensor_tensor).
    """
    return "A" if pair == 0 else "D"


def _build_nc():
    f32 = mybir.dt.float32
    bf16 = mybir.dt.bfloat16
    i16 = mybir.dt.int16
    EXP = mybir.ActivationFunctionType.Exp
    IDENT = mybir.ActivationFunctionType.Identity
    ADD = mybir.AluOpType.add
    MULT = mybir.AluOpType.mult

    nc = bacc.Bacc("TRN2", target_bir_lowering=False, debug=False)

    def din(name, shape, dt):
        return nc.dram_tensor(name, shape, dt, kind="ExternalInput").ap()

    x_bf = din("x_bf", [128, 2, 2, 512], bf16)  # [p, tchunk, chalf, t]
    wk = din("wk", [128, 2, 256], bf16)
    wq = din("wq", [128, 2, 256], bf16)  # pre-scaled by 1/sqrt(D)
    wv = din("wv", [128, 2, 256], bf16)
    bjt = din("bjt", [4, 128], f32)    # bk0, bk1, bq0, bq1 (q pre-scaled)
    bm2 = din("bm2", [2, 128], f32)    # b_mix + bv @ w_mix
    mbf = din("mbf", [128, 512], f32)    # col<128: B - 60*A if s>t else B; else B
    selp = din("selp", [128, 128], bf16)  # denom-row broadcast selector
    wmp = din("wmp", [128, 4, 256], bf16)  # po-row layout mix weights
    y = nc.dram_tensor("y", [C, T], bf16, kind="ExternalOutput").ap()

    with tile.TileContext(nc) as tc:
        with (
            tc.tile_pool(name="consts", bufs=1) as consts,
            tc.tile_pool(name="sb", bufs=4) as sb,
            tc.tile_pool(name="lpp", bufs=3, space="PSUM") as lp_pool,
            tc.tile_pool(name="pop", bufs=2, space="PSUM") as po_pool,
        ):
            # ---------------- input DMAs (critical-path order) -------------
            # xb[p, tc, a, t]: per-partition 4KB contiguous lines in DRAM.
            # tc0 split into 4 chunks so the (column-chunked) first
            # projections start as early as possible; rings ordered by
            # deadline (3 rings share ~150GB/s while all active).
            xb = consts.tile([128, 2, 2, 512], bf16)
            nc.sync.dma_start(out=xb[:, 0, 0, 0:256], in_=x_bf[:, 0, 0, 0:256])
            nc.sync.dma_start(out=xb[:, 0, 1, 0:256], in_=x_bf[:, 0, 1, 0:256])
            nc.sync.dma_start(out=xb[:, 0, 0, 256:512],
                              in_=x_bf[:, 0, 0, 256:512])
            nc.sync.dma_start(out=xb[:, 0, 1, 256:512],
                              in_=x_bf[:, 0, 1, 256:512])
            w_tiles = {}
            for name in ("wk", "wq", "wv"):
                w_tiles[name] = consts.tile([128, 2, 256], bf16,
                                            tag=name, name=name)
            nc.sync.dma_start(out=w_tiles["wv"], in_=wv)
            # wk j0 rides the scalar HWDGE queue (lower latency, idle early)
            # so the first projection isn't gated on the big gpsimd loads.
            # bjt AFTER it: its 128 tiny descriptors would block the ring.
            nc.scalar.dma_start(out=w_tiles["wk"][:, 0, 0:128],
                                in_=wk[:, 0, 0:128])
            nc.scalar.dma_start(out=w_tiles["wk"][:, 1, 0:128],
                                in_=wk[:, 1, 0:128])
            bjt_sb = consts.tile([128, 4], f32)
            nc.scalar.dma_start(out=bjt_sb, in_=bjt.rearrange("a p -> p a"))
            nc.gpsimd.dma_start(out=w_tiles["wq"][:, :, 0:128],
                                in_=wq[:, :, 0:128])
            nc.gpsimd.dma_start(out=w_tiles["wk"][:, :, 128:256],
                                in_=wk[:, :, 128:256])
            nc.gpsimd.dma_start(out=w_tiles["wq"][:, :, 128:256],
                                in_=wq[:, :, 128:256])
            mbf_sb = consts.tile([128, 512], f32)
            nc.scalar.dma_start(out=mbf_sb, in_=mbf)
            bm2_sb = consts.tile([128, 2], f32)
            nc.scalar.dma_start(out=bm2_sb, in_=bm2.rearrange("a p -> p a"))
            nc.sync.dma_start(out=xb[:, 1], in_=x_bf[:, 1])
            selp_sb = consts.tile([128, 128], bf16)
            nc.sync.dma_start(out=selp_sb, in_=selp)
            wmp_sb = consts.tile([128, 4, 256], bf16)
            nc.sync.dma_start(out=wmp_sb, in_=wmp)

            # warm the exp table before attention needs it
            warm = consts.tile([128, 1], f32, tag="warm", name="warm")
            nc.scalar.activation(out=warm, in_=bjt_sb[:, 0:1], func=EXP)

            qkT = consts.tile([128, 4, T], bf16)  # 0-1: kT g0/g1, 2-3: qT
            vsb = consts.tile([128, 8, H, 33], bf16)  # [p, st, head, V|1]
            nc.vector.memset(vsb[:, :, :, 32:33], 1.0)
            attnT = consts.tile([128, 4, T], bf16)    # [po-row, 2g+pair, t]

            # ---------------- projections --------------------------------
            def proj_group(dst, wname, jl, tch, mover, chunked=False):
                w_s = w_tiles[wname]
                js = slice(jl * 128, (jl + 1) * 128)
                ts_ = slice(tch * 512, (tch + 1) * 512)
                p = lp_pool.tile([128, 2, 512], f32, tag="lp", name="pp")[:, 0, :]
                if chunked:
                    # column-chunked so the first matmuls can start before
                    # the whole x t-half has landed
                    for ch in range(2):
                        cc = slice(256 * ch, 256 * (ch + 1))
                        nc.tensor.matmul(out=p[:, cc], lhsT=w_s[:, 0, js],
                                         rhs=xb[:, tch, 0, cc],
                                         start=True, stop=False,
                                         skip_group_check=True)
                        nc.tensor.matmul(out=p[:, cc], lhsT=w_s[:, 1, js],
                                         rhs=xb[:, tch, 1, cc],
                                         start=False, stop=True,
                                         skip_group_check=True)
                else:
                    nc.tensor.matmul(out=p, lhsT=w_s[:, 0, js],
                                     rhs=xb[:, tch, 0],
                                     start=True, stop=False)
                    nc.tensor.matmul(out=p, lhsT=w_s[:, 1, js],
                                     rhs=xb[:, tch, 1],
                                     start=False, stop=True)
                if mover == "A":
                    nc.scalar.activation(out=qkT[:, dst, ts_], in_=p,
                                         func=IDENT,
                                         bias=bjt_sb[:, dst:dst + 1])
                else:
                    eng = nc.vector if mover == "D" else nc.gpsimd
                    eng.tensor_scalar(out=qkT[:, dst, ts_], in0=p,
                                      scalar1=bjt_sb[:, dst:dst + 1],
                                      scalar2=None, op0=ADD)

            def v_group(st, mover):
                tc, tr = st // 4, st % 4
                ss = slice(tr * 128, (tr + 1) * 128)
                p = lp_pool.tile([128, 2, 512], f32, tag="lp",
                                 name="pv")[:, 0, 0:256]
                nc.tensor.matmul(out=p, lhsT=xb[:, tc, 0, ss],
                                 rhs=w_tiles["wv"][:, 0, :],
                                 start=True, stop=False)
                nc.tensor.matmul(out=p, lhsT=xb[:, tc, 1, ss],
                                 rhs=w_tiles["wv"][:, 1, :],
                                 start=False, stop=True)
                src = p.rearrange("p (h e) -> p h e", e=32)
                dstv = vsb[:, st, :, 0:32]
                if mover == "A":
                    nc.scalar.copy(out=dstv, in_=src)
                elif mover == "D":
                    nc.vector.tensor_copy(out=dstv, in_=src)
                else:
                    nc.gpsimd.tensor_copy(out=dstv, in_=src)

            # head phase: only what c0/g0 needs
            proj_group(0, "wk", 0, 0, "A", chunked=True)   # kT g0, t 0:512
            proj_group(2, "wq", 0, 0, "A", chunked=True)   # qT g0, t 0:512
            for st in range(4):
                v_group(st, "A" if st % 2 == 0 else "D")

            # woven work lists: (c, g, st) -> list of thunks
            weave = {}

            def add_weave(c, g, st, fn):
                weave.setdefault((c, g, st), []).append(fn)

            add_weave(0, 0, 0, lambda: proj_group(1, "wk", 1, 0, "A"))
            add_weave(0, 0, 1, lambda: proj_group(3, "wq", 1, 0, "A"))
            add_weave(0, 0, 2, lambda: proj_group(0, "wk", 0, 1, "A"))
            add_weave(0, 0, 3, lambda: proj_group(2, "wq", 0, 1, "A"))
            add_weave(0, 1, 0, lambda: proj_group(1, "wk", 1, 1, "A"))
            add_weave(0, 1, 1, lambda: proj_group(3, "wq", 1, 1, "A"))
            for st in range(4, 8):
                m = "A" if st % 2 == 0 else "D"
                add_weave(0, 1, st - 4, lambda st=st, m=m: v_group(st, m))

            # ---------------- attention ----------------------------------
            ouPO = {}   # (c, g) -> [128, 2, 512] bf16 sbuf

            def drain_po(c, g, po, q0=0, q1=512):
                if (c, g) not in ouPO:
                    ouPO[(c, g)] = sb.tile([128, 2, 512], bf16, tag="oup",
                                           name=f"ou{c}{g}", bufs=4)
                t_ = ouPO[(c, g)]
                nc.vector.tensor_copy(out=t_[:, 0, q0:q1],
                                      in_=po[0][:, q0:q1])
                nc.scalar.copy(out=t_[:, 1, q0:q1], in_=po[1][:, q0:q1])

            def div_cg(c, g, q0=0, q1=512):
                # broadcast denominator rows (32/96) to all partitions via
                # the selector matmul (one per pair/bank), reciprocal on all
                # 128 lanes, then one tensor_mul for both pairs.
                w = q1 - q0
                bc = lp_pool.tile([128, 2, 512], f32, tag="lp", name="bc")
                if w <= 128:
                    # both pairs fit one PSUM bank: single broadcast matmul
                    nc.tensor.matmul(out=bc[:, :, q0:q1], lhsT=selp_sb,
                                     rhs=ouPO[(c, g)][:, :, q0:q1],
                                     start=True, stop=True,
                                     skip_group_check=True)
                else:
                    for pair in range(2):
                        nc.tensor.matmul(out=bc[:, pair, q0:q1],
                                         lhsT=selp_sb,
                                         rhs=ouPO[(c, g)][:, pair, q0:q1],
                                         start=True, stop=True,
                                         skip_group_check=True)
                rcf = sb.tile([128, 2, 512], f32, tag="rcf", name="rcf",
                              bufs=4)
                nc.vector.reciprocal_approx_fast(
                    out=rcf[:, :, 0:w], in_=bc[:, :, q0:q1])
                nc.vector.tensor_mul(
                    out=attnT[:, 2 * g:2 * g + 2, c * 512 + q0:c * 512 + q1],
                    in0=ouPO[(c, g)][:, :, q0:q1], in1=rcf[:, :, 0:w])

            mp_held = {}

            def mix_part(c, c2t, gps, start, stop, q0=0, q1=512):
                cs = slice(c * 512 + q0, c * 512 + q1)
                c2s = slice(c2t * 128, (c2t + 1) * 128)
                if (c, c2t) not in mp_held:
                    mp_held[(c, c2t)] = lp_pool.tile(
                        [128, 2, 512], f32, tag="lp", name="mp")[:, 0, :]
                mp = mp_held[(c, c2t)]
                for i, gp in enumerate(gps):
                    nc.tensor.matmul(out=mp[:, q0:q1],
                                     lhsT=wmp_sb[:, gp, c2s],
                                     rhs=attnT[:, gp, cs],
                                     start=(start and i == 0),
                                     stop=(stop and i == len(gps) - 1),
                                     skip_group_check=True)

            def mix_fin(c, c2t, q0=0, q1=512, eng="A"):
                mp = mp_held[(c, c2t)]
                os_ = sb.tile([128, 512], bf16, tag="os", name="os")
                nc.vector.scalar_tensor_tensor(
                    out=os_[:, q0:q1], in0=mp[:, q0:q1],
                    scalar=bm2_sb[:, c2t:c2t + 1],
                    in1=xb[:, c, c2t, q0:q1], op0=ADD, op1=ADD)
                y_r = y.rearrange("(a p) t -> p a t", p=128)
                qe1 = (nc.sync, nc.scalar)[c2t]
                qe2 = (nc.gpsimd, nc.sync)[c2t]
                if q1 - q0 >= 256:
                    qm = (q0 + q1) // 2
                    qe1.dma_start(out=y_r[:, c2t, c * 512 + q0:c * 512 + qm],
                                  in_=os_[:, q0:qm])
                    qe2.dma_start(out=y_r[:, c2t, c * 512 + qm:c * 512 + q1],
                                  in_=os_[:, qm:q1])
                else:
                    qe1.dma_start(out=y_r[:, c2t, c * 512 + q0:c * 512 + q1],
                                  in_=os_[:, q0:q1])

            def mix_c(c, c2t):
                mix_part(c, c2t, (0, 1, 2, 3), True, True)
                mix_fin(c, c2t)

            # drains + division woven two slots later than strictly needed
            # so they don't sit between the latency-critical early-group
            # exps on ACT/DVE; c0's mix woven into (c1, g1)'s late slots.
            po_of = {}
            add_weave(0, 1, 3, lambda: drain_po(0, 0, po_of[(0, 0)]))
            add_weave(1, 0, 1, lambda: div_cg(0, 0))
            add_weave(1, 0, 3, lambda: drain_po(0, 1, po_of[(0, 1)]))
            add_weave(1, 0, 5, lambda: div_cg(0, 1))
            add_weave(1, 0, 6, lambda: mix_c(0, 0))
            add_weave(1, 0, 7, lambda: mix_c(0, 1))
            add_weave(1, 1, 3, lambda: drain_po(1, 0, po_of[(1, 0)]))
            add_weave(1, 1, 5, lambda: div_cg(1, 0))
            add_weave(1, 1, 7, lambda: mix_part(1, 0, (0, 1), True, False))

            for c in range(2):
                n_st = 4 + 4 * c
                for g in range(2):
                    po = {
                        0: po_pool.tile([128, 512], f32, tag="po", name="po0"),
                        1: po_pool.tile([128, 512], f32, tag="po", name="po1"),
                    }
                    e_tiles = {}
                    LAG = 4
                    for st in range(n_st + LAG):
                        if st < n_st:
                            tlo_r = 128 * st - 512 * c
                            diag = tlo_r >= 0
                            tlo = max(tlo_r, 0)
                            for pair in (1, 0):
                                eng = _unit_engine(c, g, st, pair)
                                lp = lp_pool.tile([128, 2, 512], f32,
                                                  tag="lp", name="lp")
                                for h2 in range(2):
                                    hl = 2 * pair + h2
                                    rp = 32 * hl
                                    kT_l = qkT[rp:rp + 32, g,
                                               st * 128:(st + 1) * 128]
                                    qg = qkT[rp:rp + 32, 2 + g, :]
                                    nc.tensor.matmul(
                                        out=lp[:, h2, tlo:512], lhsT=kT_l,
                                        rhs=qg[:, c * 512 + tlo:(c + 1) * 512],
                                        start=True, stop=True,
                                        tile_position=(rp, 0),
                                    )
                                E = sb.tile([128, 2, 512], bf16, tag="E",
                                            name="E", bufs=10)
                                if eng == "A":
                                    nc.scalar.activation(
                                        out=E[:, :, tlo:512],
                                        in_=lp[:, :, tlo:512], func=EXP)
                                    if diag:
                                        # zero the s>t triangle of the diag
                                        # block: keep where (t - s) >= 0
                                        nc.gpsimd.affine_select(
                                            out=E[:, :, tlo:tlo + 128],
                                            in_=E[:, :, tlo:tlo + 128],
                                            pattern=[[0, 2], [1, 128]],
                                            compare_op=mybir.AluOpType.is_ge,
                                            fill=0.0,
                                            base=0,
                                            channel_multiplier=-1,
                                        )
                                else:
                                    E16 = E.bitcast(i16)
                                    if diag:
                                        w_ = 512 - tlo
                                        mb_b = bass.AP(
                                            tensor=mbf_sb.tensor,
                                            offset=mbf_sb.offset,
                                            ap=[list(mbf_sb.ap[0]), [0, 2],
                                                [1, w_]],
                                        )
                                        nc.vector.scalar_tensor_tensor(
                                            out=E16[:, :, tlo:512],
                                            in0=lp[:, :, tlo:512],
                                            scalar=EXP_A, in1=mb_b,
                                            op0=MULT, op1=ADD)
                                    else:
                                        nc.vector.tensor_scalar(
                                            out=E16[:, :, tlo:512],
                                            in0=lp[:, :, tlo:512],
                                            scalar1=EXP_A, scalar2=EXP_B,
                                            op0=MULT, op1=ADD)
                                e_tiles[(st, pair)] = E
                            for fn in weave.get((c, g, st), []):
                                fn()
                        if st >= LAG:
                            stp = st - LAG
                            tlo_p = max(128 * stp - 512 * c, 0)
                            for pair in (1, 0):
                                E = e_tiles.pop((stp, pair))
                                for h2 in range(2):
                                    hl = 2 * pair + h2
                                    h = 4 * g + hl
                                    nc.tensor.matmul(
                                        out=po[pair][64 * h2:64 * h2 + 33,
                                                     tlo_p:512],
                                        lhsT=vsb[:, stp, h, :],
                                        rhs=E[:, h2, tlo_p:512],
                                        start=(stp == 0),
                                        stop=(stp == n_st - 1),
                                        skip_group_check=True,
                                        tile_position=(0, 64 * h2),
                                    )
                    po_of[(c, g)] = po

            # ---------------- tail: c1g1 division + c1 mix ---------------
            # AV st7 only touches query cols 384:512 of the (1,1) window, so
            # cols 0:384 ("A") drain/divide/mix while st7 is still in
            # flight; only the 128-wide "B" chunk rides the critical tail.
            mix_part(1, 1, (0, 1), True, False)
            drain_po(1, 1, po_of[(1, 1)], 0, 384)
            div_cg(1, 1, 0, 384)
            mix_part(1, 0, (2, 3), False, False, 0, 384)
            mix_part(1, 1, (2, 3), False, False, 0, 384)
            mix_fin(1, 0, 0, 384)
            mix_fin(1, 1, 0, 384)
            drain_po(1, 1, po_of[(1, 1)], 384, 512)
            div_cg(1, 1, 384, 512)
            mix_part(1, 0, (2, 3), False, True, 384, 512)
            mix_part(1, 1, (2, 3), False, True, 384, 512)
            mix_fin(1, 0, 384, 512, eng="D")
            mix_fin(1, 1, 384, 512, eng="D")

    nc.compile()
    return nc


def _host_inputs(image, w_kqv, b_kqv, w_mix, b_mix):
    s = np.float32(1.0 / np.sqrt(D))
    wk = w_kqv[:, :256]
    wq = w_kqv[:, 256:512] * s
    wv = w_kqv[:, 512:]
    bk = b_kqv[:256].astype(np.float32)
    bq = (b_kqv[256:512] * s).astype(np.float32)
    bv = b_kqv[512:].astype(np.float32)
    bjt = np.stack([bk[0:128], bk[128:256], bq[0:128], bq[128:256]])
    bm_eff = (np.asarray(b_mix, np.float32)
              + bv @ np.asarray(w_mix, np.float32))
    bm2 = bm_eff.reshape(2, 128)

    idx = np.arange(128)
    mask = (idx[:, None] > idx[None, :]).astype(np.float32)  # s > t
    mbf = np.full((128, 512), EXP_B, np.float32)
    mbf[:, 0:128] = EXP_B - (MASK60 * EXP_A) * mask

    # selp[po_row, m]: broadcast denominator row 32 (h2=0) to partitions
    # 0-63, row 96 (h2=1) to partitions 64-127.
    selp = np.zeros((128, 128), np.float32)
    selp[32, 0:64] = 1.0
    selp[96, 64:128] = 1.0
    # wmp[po_row, gp, out_chan]
    wmp = np.zeros((128, 4, 256), np.float32)
    wm = np.asarray(w_mix, np.float32)
    for g in range(2):
        for pair in range(2):
            gp = 2 * g + pair
            for h2 in range(2):
                in0 = 128 * g + 32 * (2 * pair + h2)
                wmp[64 * h2:64 * h2 + 32, gp, :] = wm[in0:in0 + 32, :]

    def play(w):  # [C, 256] -> [p, a, j] contiguous
        return np.ascontiguousarray(
            np.asarray(w, np.float32).reshape(2, 128, 256).transpose(1, 0, 2)
        ).astype(BF)

    common = {
        "wk": play(wk),
        "wq": play(wq),
        "wv": play(wv),
        "wmp": wmp.astype(BF),
        "bjt": np.ascontiguousarray(bjt),
        "bm2": np.ascontiguousarray(bm2),
        "mbf": mbf,
        "selp": selp.astype(BF),
    }
    in_maps = []
    for i in range(N_CORES):
        x = np.asarray(image[i].reshape(C, T), np.float32)
        # [p, tc, a, t]: per-partition 4KB contiguous DMA lines
        x4 = np.ascontiguousarray(
            x.reshape(2, 128, 2, 512).transpose(1, 2, 0, 3)).astype(BF)
        in_maps.append({**common, "x_bf": x4})
    return in_maps


def _run(inputs, trace=False):
    if "nc" not in _CACHE:
        _CACHE["nc"] = _build_nc()
    nc = _CACHE["nc"]
    in_maps = _host_inputs(
        np.asarray(inputs["image"], np.float32),
        np.asarray(inputs["w_kqv"], np.float32),
        np.asarray(inputs["b_kqv"], np.float32),
        np.asarray(inputs["w_mix"], np.float32),
        np.asarray(inputs["b_mix"], np.float32),
    )
    res = run_bass_kernel_spmd(nc, in_maps, list(range(N_CORES)), trace=trace)
    out = np.stack(
        [np.asarray(res.results[i]["y"]).reshape(C, 32, 32) for i in range(N_CORES)]
    ).astype(np.float32)
    return out, res


def kernel(**inputs):
    out, _ = _run(inputs, trace=False)
    return out



# revision 70
# speedup vs baseline: 1.2184x; 1.0236x over previous
"""PixelAttention Trainium2 kernel (v2).

Data-parallel: one image per NeuronCore. Per core:
    seq  = image.reshape(C, T).T            # [T, C], T = 32*32
    kqv  = seq @ w_kqv + b_kqv
    per-head causal attention (8 heads, head_dim 32), softmax over keys
    out  = mix(attn) + b_mix + image

Key design points (vs v1 baseline at ~87us):
  - exp is split across ScalarE (true Exp) and DVE (one-instruction
    Schraudolph fast-exp: int16(L*2^7/ln2 + (127*128-5.5)) bitcast to bf16,
    max rel err ~3.3%, end-to-end ~2e-3).
  - causal diag masking: ScalarE-assigned tiles get -60 added to the masked
    triangle via an eye-matmul PSUM accumulate (PE); DVE-assigned tiles fold
    the mask into the fast-exp via scalar_tensor_tensor with a const tile.
  - AV matmuls use M=64 weights [V | ones | zeros*31] so all 128 PSUM rows
    are written (denominator at rows 32/96, junk rows exactly zero); the
    division then runs directly on the PO layout, no compaction DMAs.
  - kqv biases ride on the psum->sbuf mover ops; v-bias is folded into the
    mix bias host-side (bm_eff = b_mix + bv @ w_mix).
  - software pipeline: logits(st) issue before AV(st-1) so the PE never
    stalls on exp and stays at full pstate.
"""

import numpy as np
import ml_dtypes

import concourse.bass as bass
import concourse.tile as tile
from concourse import bacc, mybir
from concourse.bass_utils import run_bass_kernel_spmd

BF = ml_dtypes.bfloat16
T, C, H, D = 1024, 256, 8, 32
N_CORES = 8

EXP_A = float(2.0**7 / np.log(2.0))        # 184.66496
EXP_B = float(127 * 128 - 5.5)             # 16250.5
MASK60 = 60.0                              # causal mask additive offset

_CACHE = {}


def _unit_engine(c, g, st, pair):
    """Which engine computes exp for unit (c, g, st, pair).

    'A' = ScalarE true exp (diag tiles masked by Pool tri-multiply on E),
    'D' = DVE fast-exp (diag mask folded into scalar_tensor_tensor).
    """
    return "A" if pair == 0 else "D"


def _build_nc():
    f32 = mybir.dt.float32
    bf16 = mybir.dt.bfloat16
    i16 = mybir.dt.int16
    EXP = mybir.ActivationFunctionType.Exp
    IDENT = mybir.ActivationFunctionType.Identity
    ADD = mybir.AluOpType.add
    MULT = mybir.AluOpType.mult

    nc = bacc.Bacc("TRN2", target_bir_lowering=False, debug=False)

    def din(name, shape, dt):
        return nc.dram_tensor(name, shape, dt, kind="ExternalInput").ap()

    x_bf = din("x_bf", [128, 2, 2, 512], bf16)  # [p, tchunk, chalf, t]
    wk = din("wk", [128, 2, 256], bf16)
    wq = din("wq", [128, 2, 256], bf16)  # pre-scaled by 1/sqrt(D)
    wv = din("wv", [128, 2, 256], bf16)
    bjt = din("bjt", [4, 128], f32)    # bk0, bk1, bq0, bq1 (q pre-scaled)
    bm2 = din("bm2", [2, 128], f32)    # b_mix + bv @ w_mix
    mbf = din("mbf", [128, 512], f32)    # col<128: B - 60*A if s>t else B; else B
    selp = din("selp", [128, 128], bf16)  # denom-row broadcast selector
    wmp = din("wmp", [128, 4, 256], bf16)  # po-row layout mix weights
    y = nc.dram_tensor("y", [C, T], bf16, kind="ExternalOutput").ap()

    with tile.TileContext(nc) as tc:
        with (
            tc.tile_pool(name="consts", bufs=1) as consts,
            tc.tile_pool(name="sb", bufs=4) as sb,
            tc.tile_pool(name="lpp", bufs=3, space="PSUM") as lp_pool,
            tc.tile_pool(name="pop", bufs=2, space="PSUM") as po_pool,
        ):
            # ---------------- input DMAs (critical-path order) -------------
            # xb[p, tc, a, t]: per-partition 4KB contiguous lines in DRAM.
            # tc0 split into 4 chunks so the (column-chunked) first
            # projections start as early as possible; rings ordered by
            # deadline (3 rings share ~150GB/s while all active).
            xb = consts.tile([128, 2, 2, 512], bf16)
            nc.sync.dma_start(out=xb[:, 0, 0, 0:256], in_=x_bf[:, 0, 0, 0:256])
            nc.sync.dma_start(out=xb[:, 0, 1, 0:256], in_=x_bf[:, 0, 1, 0:256])
            nc.sync.dma_start(out=xb[:, 0, 0, 256:512],
                              in_=x_bf[:, 0, 0, 256:512])
            nc.sync.dma_start(out=xb[:, 0, 1, 256:512],
                              in_=x_bf[:, 0, 1, 256:512])
            w_tiles = {}
            for name in ("wk", "wq", "wv"):
                w_tiles[name] = consts.tile([128, 2, 256], bf16,
                                            tag=name, name=name)
            nc.sync.dma_start(out=w_tiles["wv"], in_=wv)
            # wk j0 rides the scalar HWDGE queue (lower latency, idle early)
            # so the first projection isn't gated on the big gpsimd loads.
            # bjt AFTER it: its 128 tiny descriptors would block the ring.
            nc.scalar.dma_start(out=w_tiles["wk"][:, 0, 0:128],
                                in_=wk[:, 0, 0:128])
            nc.scalar.dma_start(out=w_tiles["wk"][:, 1, 0:128],
                                in_=wk[:, 1, 0:128])
            bjt_sb = consts.tile([128, 4], f32)
            nc.scalar.dma_start(out=bjt_sb, in_=bjt.rearrange("a p -> p a"))
            nc.gpsimd.dma_start(out=w_tiles["wq"][:, :, 0:128],
                                in_=wq[:, :, 0:128])
            nc.gpsimd.dma_start(out=w_tiles["wk"][:, :, 128:256],
                                in_=wk[:, :, 128:256])
            nc.gpsimd.dma_start(out=w_tiles["wq"][:, :, 128:256],
                                in_=wq[:, :, 128:256])
            mbf_sb = consts.tile([128, 512], f32)
            nc.scalar.dma_start(out=mbf_sb, in_=mbf)
            bm2_sb = consts.tile([128, 2], f32)
            nc.scalar.dma_start(out=bm2_sb, in_=bm2.rearrange("a p -> p a"))
            nc.sync.dma_start(out=xb[:, 1], in_=x_bf[:, 1])
            selp_sb = consts.tile([128, 128], bf16)
            nc.sync.dma_start(out=selp_sb, in_=selp)
            wmp_sb = consts.tile([128, 4, 256], bf16)
            nc.sync.dma_start(out=wmp_sb, in_=wmp)

            # warm the exp table before attention needs it
            warm = consts.tile([128, 1], f32, tag="warm", name="warm")
            nc.scalar.activation(out=warm, in_=bjt_sb[:, 0:1], func=EXP)

            qkT = consts.tile([128, 4, T], bf16)  # 0-1: kT g0/g1, 2-3: qT
            vsb = consts.tile([128, 8, H, 33], bf16)  # [p, st, head, V|1]
            nc.vector.memset(vsb[:, :, :, 32:33], 1.0)
            attnT = consts.tile([128, 4, T], bf16)    # [po-row, 2g+pair, t]

            # ---------------- projections --------------------------------
            def proj_group(dst, wname, jl, tch, mover, chunked=False):
                w_s = w_tiles[wname]
                js = slice(jl * 128, (jl + 1) * 128)
                ts_ = slice(tch * 512, (tch + 1) * 512)
                p = lp_pool.tile([128, 2, 512], f32, tag="lp", name="pp")[:, 0, :]
                if chunked:
                    # column-chunked so the first matmuls can start before
                    # the whole x t-half has landed
                    for ch in range(2):
                        cc = slice(256 * ch, 256 * (ch + 1))
                        nc.tensor.matmul(out=p[:, cc], lhsT=w_s[:, 0, js],
                                         rhs=xb[:, tch, 0, cc],
                                         start=True, stop=False,
                                         skip_group_check=True)
                        nc.tensor.matmul(out=p[:, cc], lhsT=w_s[:, 1, js],
                                         rhs=xb[:, tch, 1, cc],
                                         start=False, stop=True,
                                         skip_group_check=True)
                else:
                    nc.tensor.matmul(out=p, lhsT=w_s[:, 0, js],
                                     rhs=xb[:, tch, 0],
                                     start=True, stop=False)
                    nc.tensor.matmul(out=p, lhsT=w_s[:, 1, js],
                                     rhs=xb[:, tch, 1],
                                     start=False, stop=True)
                if mover == "A":
                    nc.scalar.activation(out=qkT[:, dst, ts_], in_=p,
                                         func=IDENT,
                                         bias=bjt_sb[:, dst:dst + 1])
                else:
                    eng = nc.vector if mover == "D" else nc.gpsimd
                    eng.tensor_scalar(out=qkT[:, dst, ts_], in0=p,
                                      scalar1=bjt_sb[:, dst:dst + 1],
                                      scalar2=None, op0=ADD)

            def v_group(st, mover):
                tc, tr = st // 4, st % 4
                ss = slice(tr * 128, (tr + 1) * 128)
                p = lp_pool.tile([128, 2, 512], f32, tag="lp",
                                 name="pv")[:, 0, 0:256]
                nc.tensor.matmul(out=p, lhsT=xb[:, tc, 0, ss],
                                 rhs=w_tiles["wv"][:, 0, :],
                                 start=True, stop=False)
                nc.tensor.matmul(out=p, lhsT=xb[:, tc, 1, ss],
                                 rhs=w_tiles["wv"][:, 1, :],
                                 start=False, stop=True)
                src = p.rearrange("p (h e) -> p h e", e=32)
                dstv = vsb[:, st, :, 0:32]
                if mover == "A":
                    nc.scalar.copy(out=dstv, in_=src)
                elif mover == "D":
                    nc.vector.tensor_copy(out=dstv, in_=src)
                else:
                    nc.gpsimd.tensor_copy(out=dstv, in_=src)

            # head phase: only what c0/g0 needs
            proj_group(0, "wk", 0, 0, "A", chunked=True)   # kT g0, t 0:512
            proj_group(2, "wq", 0, 0, "A", chunked=True)   # qT g0, t 0:512
            for st in range(4):
                v_group(st, "A" if st % 2 == 0 else "D")

            # woven work lists: (c, g, st) -> list of thunks
            weave = {}

            def add_weave(c, g, st, fn):
                weave.setdefault((c, g, st), []).append(fn)

            add_weave(0, 0, 0, lambda: proj_group(1, "wk", 1, 0, "A"))
            add_weave(0, 0, 1, lambda: proj_group(3, "wq", 1, 0, "A"))
            add_weave(0, 0, 2, lambda: proj_group(0, "wk", 0, 1, "A"))
            add_weave(0, 0, 3, lambda: proj_group(2, "wq", 0, 1, "A"))
            add_weave(0, 1, 0, lambda: proj_group(1, "wk", 1, 1, "A"))
            add_weave(0, 1, 1, lambda: proj_group(3, "wq", 1, 1, "A"))
            for st in range(4, 8):
                m = "A" if st % 2 == 0 else "D"
                add_weave(0, 1, st - 4, lambda st=st, m=m: v_group(st, m))

            # ---------------- attention ----------------------------------
            ouPO = {}   # (c, g) -> [128, 2, 512] bf16 sbuf

            def drain_po(c, g, po, q0=0, q1=512):
                if (c, g) not in ouPO:
                    ouPO[(c, g)] = sb.tile([128, 2, 512], bf16, tag="oup",
                                           name=f"ou{c}{g}", bufs=4)
                t_ = ouPO[(c, g)]
                nc.vector.tensor_copy(out=t_[:, 0, q0:q1],
                                      in_=po[0][:, q0:q1])
                nc.scalar.copy(out=t_[:, 1, q0:q1], in_=po[1][:, q0:q1])

            def div_cg(c, g, q0=0, q1=512):
                # broadcast denominator rows (32/96) to all partitions via
                # the selector matmul (one per pair/bank), reciprocal on all
                # 128 lanes, then one tensor_mul for both pairs.
                w = q1 - q0
                bc = lp_pool.tile([128, 2, 512], f32, tag="lp", name="bc")
                if w <= 128:
                    # both pairs fit one PSUM bank: single broadcast matmul
                    nc.tensor.matmul(out=bc[:, :, q0:q1], lhsT=selp_sb,
                                     rhs=ouPO[(c, g)][:, :, q0:q1],
                                     start=True, stop=True,
                                     skip_group_check=True)
                else:
                    for pair in range(2):
                        nc.tensor.matmul(out=bc[:, pair, q0:q1],
                                         lhsT=selp_sb,
                                         rhs=ouPO[(c, g)][:, pair, q0:q1],
                                         start=True, stop=True,
                                         skip_group_check=True)
                rcf = sb.tile([128, 2, 512], f32, tag="rcf", name="rcf",
                              bufs=4)
                nc.vector.reciprocal_approx_fast(
                    out=rcf[:, :, 0:w], in_=bc[:, :, q0:q1])
                nc.vector.tensor_mul(
                    out=attnT[:, 2 * g:2 * g + 2, c * 512 + q0:c * 512 + q1],
                    in0=ouPO[(c, g)][:, :, q0:q1], in1=rcf[:, :, 0:w])

            mp_held = {}

            def mix_part(c, c2t, gps, start, stop, q0=0, q1=512):
                cs = slice(c * 512 + q0, c * 512 + q1)
                c2s = slice(c2t * 128, (c2t + 1) * 128)
                if (c, c2t) not in mp_held:
                    mp_held[(c, c2t)] = lp_pool.tile(
                        [128, 2, 512], f32, tag="lp", name="mp")[:, 0, :]
                mp = mp_held[(c, c2t)]
                for i, gp in enumerate(gps):
                    nc.tensor.matmul(out=mp[:, q0:q1],
                                     lhsT=wmp_sb[:, gp, c2s],
                                     rhs=attnT[:, gp, cs],
                                     start=(start and i == 0),
                                     stop=(stop and i == len(gps) - 1),
                                     skip_group_check=True)

            def mix_fin(c, c2t, q0=0, q1=512, eng="A"):
                mp = mp_held[(c, c2t)]
                os_ = sb.tile([128, 512], bf16, tag="os", name="os")
                nc.vector.scalar_tensor_tensor(
                    out=os_[:, q0:q1], in0=mp[:, q0:q1],
                    scalar=bm2_sb[:, c2t:c2t + 1],
                    in1=xb[:, c, c2t, q0:q1], op0=ADD, op1=ADD)
                y_r = y.rearrange("(a p) t -> p a t", p=128)
                qe1 = (nc.sync, nc.scalar)[c2t]
                qe2 = (nc.gpsimd, nc.sync)[c2t]
                if q1 - q0 >= 256:
                    qm = (q0 + q1) // 2
                    qe1.dma_start(out=y_r[:, c2t, c * 512 + q0:c * 512 + qm],
                                  in_=os_[:, q0:qm])
                    qe2.dma_start(out=y_r[:, c2t, c * 512 + qm:c * 512 + q1],
                                  in_=os_[:, qm:q1])
                else:
                    qe1.dma_start(out=y_r[:, c2t, c * 512 + q0:c * 512 + q1],
                                  in_=os_[:, q0:q1])

            def mix_c(c, c2t):
                mix_part(c, c2t, (0, 1, 2, 3), True, True)
                mix_fin(c, c2t)

            # drains + division woven two slots later than strictly needed
            # so they don't sit between the latency-critical early-group
            # exps on ACT/DVE; c0's mix woven into (c1, g1)'s late slots.
            po_of = {}
            add_weave(0, 1, 3, lambda: drain_po(0, 0, po_of[(0, 0)]))
            add_weave(1, 0, 1, lambda: div_cg(0, 0))
            add_weave(1, 0, 3, lambda: drain_po(0, 1, po_of[(0, 1)]))
            add_weave(1, 0, 5, lambda: div_cg(0, 1))
            add_weave(1, 0, 6, lambda: mix_c(0, 0))
            add_weave(1, 0, 7, lambda: mix_c(0, 1))
            add_weave(1, 1, 3, lambda: drain_po(1, 0, po_of[(1, 0)]))
            add_weave(1, 1, 5, lambda: div_cg(1, 0))
            add_weave(1, 1, 7, lambda: mix_part(1, 0, (0, 1), True, False))

            for c in range(2):
                n_st = 4 + 4 * c
                for g in range(2):
                    po = {
                        0: po_pool.tile([128, 512], f32, tag="po", name="po0"),
                        1: po_pool.tile([128, 512], f32, tag="po", name="po1"),
                    }
                    e_tiles = {}
                    LAG = 5
                    for st in range(n_st + LAG):
                        if st < n_st:
                            tlo_r = 128 * st - 512 * c
                            diag = tlo_r >= 0
                            tlo = max(tlo_r, 0)
                            for pair in (1, 0):
                                eng = _unit_engine(c, g, st, pair)
                                lp = lp_pool.tile([128, 2, 512], f32,
                                                  tag="lp", name="lp")
                                for h2 in range(2):
                                    hl = 2 * pair + h2
                                    rp = 32 * hl
                                    kT_l = qkT[rp:rp + 32, g,
                                               st * 128:(st + 1) * 128]
                                    qg = qkT[rp:rp + 32, 2 + g, :]
                                    nc.tensor.matmul(
                                        out=lp[:, h2, tlo:512], lhsT=kT_l,
                                        rhs=qg[:, c * 512 + tlo:(c + 1) * 512],
                                        start=True, stop=True,
                                        tile_position=(rp, 0),
                                    )
                                E = sb.tile([128, 2, 512], bf16, tag="E",
                                            name="E", bufs=12)
                                if eng == "A":
                                    nc.scalar.activation(
                                        out=E[:, :, tlo:512],
                                        in_=lp[:, :, tlo:512], func=EXP)
                                    if diag:
                                        # zero the s>t triangle of the diag
                                        # block: keep where (t - s) >= 0
                                        nc.gpsimd.affine_select(
                                            out=E[:, :, tlo:tlo + 128],
                                            in_=E[:, :, tlo:tlo + 128],
                                            pattern=[[0, 2], [1, 128]],
                                            compare_op=mybir.AluOpType.is_ge,
                                            fill=0.0,
                                            base=0,
                                            channel_multiplier=-1,
                                        )
                                else:
                                    E16 = E.bitcast(i16)
                                    if diag:
                                        w_ = 512 - tlo
                                        mb_b = bass.AP(
                                            tensor=mbf_sb.tensor,
                                            offset=mbf_sb.offset,
                                            ap=[list(mbf_sb.ap[0]), [0, 2],
                                                [1, w_]],
                                        )
                                        nc.vector.scalar_tensor_tensor(
                                            out=E16[:, :, tlo:512],
                                            in0=lp[:, :, tlo:512],
                                            scalar=EXP_A, in1=mb_b,
                                            op0=MULT, op1=ADD)
                                    else:
                                        nc.vector.tensor_scalar(
                                            out=E16[:, :, tlo:512],
                                            in0=lp[:, :, tlo:512],
                                            scalar1=EXP_A, scalar2=EXP_B,
                                            op0=MULT, op1=ADD)
                                e_tiles[(st, pair)] = E
                            for fn in weave.get((c, g, st), []):
                                fn()
                        if st >= LAG:
                            stp = st - LAG
                            tlo_p = max(128 * stp - 512 * c, 0)
                            for pair in (1, 0):
                                E = e_tiles.pop((stp, pair))
                                for h2 in range(2):
                                    hl = 2 * pair + h2
                                    h = 4 * g + hl
                                    nc.tensor.matmul(
                                        out=po[pair][64 * h2:64 * h2 + 33,
                                                     tlo_p:512],
                                        lhsT=vsb[:, stp, h, :],
                                        rhs=E[:, h2, tlo_p:512],
                                        start=(stp == 0),
                                        stop=(stp == n_st - 1),
                                        skip_group_check=True,
                                        tile_position=(0, 64 * h2),
                                    )
                    po_of[(c, g)] = po

            # ---------------- tail: c1g1 division + c1 mix ---------------
            # AV st7 only touches query cols 384:512 of the (1,1) window, so
            # cols 0:384 ("A") drain/divide/mix while st7 is still in
            # flight; only the 128-wide "B" chunk rides the critical tail.
            mix_part(1, 1, (0, 1), True, False)
            drain_po(1, 1, po_of[(1, 1)], 0, 384)
            div_cg(1, 1, 0, 384)
            mix_part(1, 0, (2, 3), False, False, 0, 384)
            mix_part(1, 1, (2, 3), False, False, 0, 384)
            mix_fin(1, 0, 0, 384)
            mix_fin(1, 1, 0, 384)
            drain_po(1, 1, po_of[(1, 1)], 384, 512)
            div_cg(1, 1, 384, 512)
            mix_part(1, 0, (2, 3), False, True, 384, 512)
            mix_part(1, 1, (2, 3), False, True, 384, 512)
            mix_fin(1, 0, 384, 512, eng="D")
            mix_fin(1, 1, 384, 512, eng="D")

    nc.compile()
    return nc


def _host_inputs(image, w_kqv, b_kqv, w_mix, b_mix):
    s = np.float32(1.0 / np.sqrt(D))
    wk = w_kqv[:, :256]
    wq = w_kqv[:, 256:512] * s
    wv = w_kqv[:, 512:]
    bk = b_kqv[:256].astype(np.float32)
    bq = (b_kqv[256:512] * s).astype(np.float32)
    bv = b_kqv[512:].astype(np.float32)
    bjt = np.stack([bk[0:128], bk[128:256], bq[0:128], bq[128:256]])
    bm_eff = (np.asarray(b_mix, np.float32)
              + bv @ np.asarray(w_mix, np.float32))
    bm2 = bm_eff.reshape(2, 128)

    idx = np.arange(128)
    mask = (idx[:, None] > idx[None, :]).astype(np.float32)  # s > t
    mbf = np.full((128, 512), EXP_B, np.float32)
    mbf[:, 0:128] = EXP_B - (MASK60 * EXP_A) * mask

    # selp[po_row, m]: broadcast denominator row 32 (h2=0) to partitions
    # 0-63, row 96 (h2=1) to partitions 64-127.
    selp = np.zeros((128, 128), np.float32)
    selp[32, 0:64] = 1.0
    selp[96, 64:128] = 1.0
    # wmp[po_row, gp, out_chan]
    wmp = np.zeros((128, 4, 256), np.float32)
    wm = np.asarray(w_mix, np.float32)
    for g in range(2):
        for pair in range(2):
            gp = 2 * g + pair
            for h2 in range(2):
                in0 = 128 * g + 32 * (2 * pair + h2)
                wmp[64 * h2:64 * h2 + 32, gp, :] = wm[in0:in0 + 32, :]

    def play(w):  # [C, 256] -> [p, a, j] contiguous
        return np.ascontiguousarray(
            np.asarray(w, np.float32).reshape(2, 128, 256).transpose(1, 0, 2)
        ).astype(BF)

    common = {
        "wk": play(wk),
        "wq": play(wq),
        "wv": play(wv),
        "wmp": wmp.astype(BF),
        "bjt": np.ascontiguousarray(bjt),
        "bm2": np.ascontiguousarray(bm2),
        "mbf": mbf,
        "selp": selp.astype(BF),
    }
    in_maps = []
    for i in range(N_CORES):
        x = np.asarray(image[i].reshape(C, T), np.float32)
        # [p, tc, a, t]: per-partition 4KB contiguous DMA lines
        x4 = np.ascontiguousarray(
            x.reshape(2, 128, 2, 512).transpose(1, 2, 0, 3)).astype(BF)
        in_maps.append({**common, "x_bf": x4})
    return in_maps


def _run(inputs, trace=False):
    if "nc" not in _CACHE:
        _CACHE["nc"] = _build_nc()
    nc = _CACHE["nc"]
    in_maps = _host_inputs(
        np.asarray(inputs["image"], np.float32),
        np.asarray(inputs["w_kqv"], np.float32),
        np.asarray(inputs["b_kqv"], np.float32),
        np.asarray(inputs["w_mix"], np.float32),
        np.asarray(inputs["b_mix"], np.float32),
    )
    res = run_bass_kernel_spmd(nc, in_maps, list(range(N_CORES)), trace=trace)
    out = np.stack(
        [np.asarray(res.results[i]["y"]).reshape(C, 32, 32) for i in range(N_CORES)]
    ).astype(np.float32)
    return out, res


def kernel(**inputs):
    out, _ = _run(inputs, trace=False)
    return out

